# revision 3
# baseline (speedup 1.0000x reference)
"""Trainium2 Bass kernel for BlockDiagonalACDC — level-2 factored DCT.

out = riffle(idct2(gconvD(dct2(gconvA(x))))) + bias, with both dense DCT
passes factored one radix-2 level deeper than the parity split:
  II_4096 -> bfly -> [II_2048 -> bfly -> II_1024 (+) IV_1024,
                      IV_2048 -> Givens rot -> II_1024 (+) II_1024-rev
                                 -> partition-shift combine]
Leaves are 1024^2 dense matmuls (4 per pass vs 2x 2048^2): dense PE work
halves per pass.  gconvD is conjugated into the level-2 permuted basis
(quadrant tiles with 32-sub-block diagonals, built on device from D).
The inverse pass uses C_IV^T = C_IV so its rotations also act on the
partition (frequency) side; shift-combines land on the free dim there.
Bias is injected via K=1 PSUM-init matmuls with host-solved preimage rows;
riffle is folded into the output DMA column maps.

Sharding: pure data parallel, 2048 rows per core on 8 cores.
"""

import numpy as np
import ml_dtypes

import concourse.bacc as bacc
import concourse.mybir as mybir
from concourse.tile import TileContext
from concourse.bass_utils import run_bass_kernel_spmd
from concourse.masks import make_identity

N_BATCH, D_FEAT, GROUPS = 16384, 4096, 32
N_CORES = 8
N_SHARD = N_BATCH // N_CORES      # 2048 rows per core
CHUNK = 512
N_CHUNKS = N_SHARD // CHUNK       # 4
H = 1024
H2 = 2048
ALPHA = [0, 2, 1, 3]              # k mod 4 for groups G0..G3

_BF16 = mybir.dt.bfloat16
_F32 = mybir.dt.float32
_MULT = mybir.AluOpType.mult
_ADD = mybir.AluOpType.add
_SUB = mybir.AluOpType.subtract


# ---------------------------------------------------------------------------
# host constants (validated in newconsts.py)
# ---------------------------------------------------------------------------

def _host_constants(bias):
    j1 = np.arange(H)
    II = 2.0 * np.cos(np.pi * np.arange(H)[:, None] * (2 * j1 + 1) / (2 * H))
    IV2 = 2.0 * np.cos(
        np.pi * (2 * np.arange(H)[:, None] + 1) * (2 * j1 + 1) / (4 * H))
    PHI = np.pi * (2 * np.arange(H2) + 1) / (4 * H2)
    s = 1.0 / (2.0 * D_FEAT)
    bf = ml_dtypes.bfloat16

    def lhsT4(M):   # [pj, tau, tj, pm] = M[128tau+pm, 128tj+pj]
        return np.ascontiguousarray(
            M.reshape(8, 128, 8, 128).transpose(3, 0, 2, 1)).astype(bf)

    c = {}
    c["W1"] = lhsT4(II)
    c["W1r"] = lhsT4(II[::-1, :])
    c["W2"] = lhsT4(IV2)
    IIw = II.copy()
    IIw[0, :] *= 0.5                               # diag(w) k=0
    c["W3"] = np.ascontiguousarray(
        (s * IIw).reshape(8, 128, H).transpose(1, 0, 2)).astype(bf)
    c["W2i"] = np.ascontiguousarray(
        (s * IV2).reshape(8, 128, H).transpose(1, 0, 2)).astype(bf)
    kap = np.empty(H, dtype=np.int64)
    kap[:512] = 2 * np.arange(512)
    kap[512:] = 2 * np.arange(512) + 1
    c["W4"] = np.ascontiguousarray(
        (s * II.T[kap, :]).reshape(8, 128, 8, 128).transpose(1, 0, 2, 3)
    ).astype(bf)

    # coeffs: fcoef [128, 24] = (tan|cos|nsc) x 8 fwd tiles;
    # icoef [128, 12]: tan_e, cos_e, nsc_e(=-cos_e) tiles 0..3? packed below
    jv = np.arange(H)
    fco = np.stack([np.tan(PHI[jv]), np.cos(PHI[jv]),
                    -((-1.0) ** jv) * np.cos(PHI[jv])])     # [3, 1024]
    c["fcoef"] = np.ascontiguousarray(
        fco.reshape(3, 8, 128).transpose(2, 0, 1).reshape(128, 24)
    ).astype(np.float32)                 # col = 8*which + t
    m5 = np.arange(512)
    ke, ko = 2 * m5, 2 * m5 + 1
    ico = np.stack([np.tan(PHI[ke]), np.cos(PHI[ke]), -np.cos(PHI[ke]),
                    np.tan(PHI[ko]), np.cos(PHI[ko]), np.cos(PHI[ko])])
    c["icoef"] = np.ascontiguousarray(
        ico.reshape(6, 4, 128).transpose(2, 0, 1).reshape(128, 24)
    ).astype(np.float32)                 # col = 4*which + tg

    # bias rows
    bias = np.asarray(bias, np.float64).reshape(-1)
    f = np.arange(H2)
    rcol = np.where(f % 2 == 0, f // 2, H2 + (f - 1) // 2)
    rcolr = np.where((D_FEAT - 1 - f) % 2 == 0, (D_FEAT - 1 - f) // 2,
                     H2 + (D_FEAT - 2 - f) // 2)
    biasf = bias[rcol]
    biasr = bias[rcolr]
    t_up = (biasf + biasr) / 2.0
    t_um = (biasf - biasr) / 2.0
    jj = np.arange(H)
    b_q0 = (t_up[jj] + t_up[H2 - 1 - jj]) / 2.0
    b_q1 = (t_up[jj] - t_up[H2 - 1 - jj]) / 2.0
    t_ev, t_od = t_um[0::2], t_um[1::2]
    alpha_v = np.empty(H)
    alpha_v[0] = t_ev[0]
    alpha_v[1:] = (t_od[:-1] + t_ev[1:]) / 2.0
    beta = np.zeros(H + 1)
    beta[1:H] = (t_ev[1:] - t_od[:-1]) / 2.0
    beta[H] = -t_od[H - 1]
    b_P = beta[H - np.arange(H)]
    c["brows"] = np.concatenate(
        [b_q0, b_q1, alpha_v, b_P])[None].astype(bf)   # [1, 4096]
    return c


def _neg(hi, step):
    """stop index for a negative-step slice ending at 0 inclusive."""
    return slice(hi, None, step)


def _build_program(reps=1):
    nc = bacc.Bacc()
    xs = nc.dram_tensor("xs", (N_SHARD, D_FEAT), _F32, kind="ExternalInput")
    Aw = nc.dram_tensor("Aw", (GROUPS, 128, 128), _F32, kind="ExternalInput")
    Dw = nc.dram_tensor("Dw", (GROUPS, 128, 128), _F32, kind="ExternalInput")
    W1d = nc.dram_tensor("W1d", (128, 8, 8, 128), _BF16, kind="ExternalInput")
    W1rd = nc.dram_tensor("W1rd", (128, 8, 8, 128), _BF16, kind="ExternalInput")
    W2d = nc.dram_tensor("W2d", (128, 8, 8, 128), _BF16, kind="ExternalInput")
    W3d = nc.dram_tensor("W3d", (128, 8, H), _BF16, kind="ExternalInput")
    W2id = nc.dram_tensor("W2id", (128, 8, H), _BF16, kind="ExternalInput")
    W4d = nc.dram_tensor("W4d", (128, 8, 8, 128), _BF16, kind="ExternalInput")
    fcoefd = nc.dram_tensor("fcoefd", (128, 24), _F32, kind="ExternalInput")
    icoefd = nc.dram_tensor("icoefd", (128, 24), _F32, kind="ExternalInput")
    browsd = nc.dram_tensor("browsd", (1, 4 * H), _BF16, kind="ExternalInput")
    out = nc.dram_tensor("out", (N_SHARD, D_FEAT), _F32, kind="ExternalOutput")

    with TileContext(nc) as tc:
        with (
            tc.tile_pool(name="const", bufs=1) as constp,
            tc.tile_pool(name="big", bufs=3) as bigp,
            tc.tile_pool(name="xbf", bufs=2) as xbfp,
            tc.tile_pool(name="wf", bufs=3) as wfp,
            tc.tile_pool(name="wi", bufs=3) as wip,
            tc.tile_pool(name="scr", bufs=2) as scrp,
            tc.tile_pool(name="s11", bufs=1) as s11p,
            tc.tile_pool(name="oo", bufs=4) as oop,
            tc.tile_pool(name="tp_ps", bufs=1, space="PSUM") as tpp,
            tc.tile_pool(name="mm_ps", bufs=3, space="PSUM") as mmp,
            tc.tile_pool(name="iv_ps", bufs=4, space="PSUM") as ivp,
        ):
            ident = constp.tile([128, 128], _BF16, tag="ident")
            make_identity(nc, ident[:])
            ones1 = constp.tile([1, 128], _BF16, tag="ones1")
            nc.gpsimd.memset(ones1[:], 1.0)
            zrow = constp.tile([1, 512], _BF16, tag="zrow")
            nc.gpsimd.memset(zrow[:], 0.0)
            fco = constp.tile([128, 24], _F32, tag="fco")
            ico = constp.tile([128, 24], _F32, tag="ico")
            brows = constp.tile([1, 4 * H], _BF16, tag="brows")
            nc.gpsimd.dma_start(fco[:], fcoefd[:])
            nc.gpsimd.dma_start(ico[:], icoefd[:])
            nc.gpsimd.dma_start(brows[:], browsd[:])
            W4r = constp.tile([128, 8, 8, 128], _BF16, tag="W4r")
            nc.sync.dma_start(W4r[:], W4d[:])

            # ---- A weights: AT[g] = A[g].T, partition-reversed output for
            # groups with (g//8) odd (free-reversed copy)
            AT = constp.tile([128, D_FEAT], _BF16, tag="AT")
            for hf in range(2):
                awbf = xbfp.tile([128, D_FEAT // 2], _BF16, tag="xbf")
                for gl in range(16):
                    g = hf * 16 + gl
                    nc.gpsimd.dma_start(awbf[:, gl * 128:(gl + 1) * 128], Aw[g])
                for g4 in range(4):
                    ps = tpp.tile([128, 512], _BF16, tag="tp")
                    for gg in range(4):
                        gl = g4 * 4 + gg
                        nc.tensor.transpose(
                            ps[:, gg * 128:(gg + 1) * 128],
                            awbf[:, gl * 128:(gl + 1) * 128], ident[:])
                    for gg in range(4):
                        g = hf * 16 + g4 * 4 + gg
                        sl = slice(g * 128, (g + 1) * 128)
                        if (g // 8) % 2 == 0:
                            nc.vector.tensor_copy(
                                AT[:, sl], ps[:, gg * 128:(gg + 1) * 128])
                        else:
                            nc.vector.tensor_copy(
                                AT[:, sl],
                                ps[:, (gg + 1) * 128 - 1:
                                   gg * 128 - 1 if gg else None:-1])

            # ---- conjugated D tiles: DTall[r][rp] = [128, 8 slots x 128]
            # r<2: slots 0..7 nat tau; r>=2: slots 0..3 nat tau 0..3,
            # slots 4..7 = reversed emissions (out tile tau'=slot-4, content
            # from original tau = 11-slot, consumes rhs tile 11-slot)
            DT = [[constp.tile([128, 8 * 128], _BF16, tag=f"DT{r}{rp}",
                               name=f"DT{r}{rp}")
                   for rp in range(4)] for r in range(4)]
            for r in range(4):
                for rp in range(4):
                    nc.gpsimd.memset(DT[r][rp][:], 0.0)
            dwh = [None, None]
            for hf in range(2):
                dwh[hf] = xbfp.tile([128, D_FEAT // 2], _BF16, tag="xbf",
                                    name="dwbf")
                for gl in range(16):
                    nc.gpsimd.dma_start(
                        dwh[hf][:, gl * 128:(gl + 1) * 128], Dw[hf * 16 + gl])
            for tau in range(8):
                dwbf0 = dwh[tau // 4]
                off = 512 * (tau % 4)
                for rp in range(4):
                    ps = tpp.tile([128, 512], _BF16, tag="tp")
                    nc.tensor.transpose(
                        ps[:, 0:128],
                        dwbf0[:, off + ALPHA[rp]:off + 512:4],
                        ident[:])
                    for r in range(4):
                        a0 = ALPHA[r]
                        if r < 2 or tau < 4:
                            base = tau * 128
                            for u in range(4):
                                nc.vector.tensor_copy(
                                    DT[r][rp][32 * u:32 * u + 32,
                                              base + 32 * u:base + 32 * u + 32],
                                    ps[32 * u:32 * u + 32, a0:a0 + 128:4])
                        if r >= 2 and tau >= 4:
                            base = (4 + (7 - tau)) * 128
                            for u in range(4):
                                nc.vector.tensor_copy(
                                    DT[r][rp][32 * u:32 * u + 32,
                                              base + 32 * (3 - u):
                                              base + 32 * (3 - u) + 32],
                                    ps[32 * u:32 * u + 32, a0 + 124::-4])

            rep_ctx = tc.For_i(0, reps, 1) if reps > 1 else None
            if rep_ctx is not None:
                rep_ctx.__enter__()
            for ci in range(N_CHUNKS):
                r0 = ci * CHUNK
                # ---- S0: transpose-in -> xT (big pool)
                xT = bigp.tile([128, 32 * CHUNK], _BF16, tag="set")
                for ntp in range(2):
                    for hf in range(2):
                        xbfs = []
                        for nn in range(2):
                            nt = ntp * 2 + nn
                            xbf = xbfp.tile([128, D_FEAT // 2], _BF16,
                                            tag="xbf")
                            nc.gpsimd.dma_start(
                                xbf[:], xs[r0 + nt * 128:r0 + (nt + 1) * 128,
                                           hf * 2048:(hf + 1) * 2048])
                            xbfs.append(xbf)
                        for fl in range(16):
                            fc = hf * 16 + fl
                            ps = tpp.tile([128, 512], _BF16, tag="tp")
                            for nn in range(2):
                                nc.tensor.transpose(
                                    ps[:, nn * 128:(nn + 1) * 128],
                                    xbfs[nn][:, fl * 128:(fl + 1) * 128],
                                    ident[:])
                            eng = nc.vector if fc % 2 else nc.scalar
                            (eng.tensor_copy if eng is nc.vector else eng.copy)(
                                xT[:, fc * CHUNK + ntp * 256:
                                   fc * CHUNK + ntp * 256 + 256],
                                ps[:, 0:256])
                # ---- S1: gconvA (32 MMs), identity slots
                z1 = bigp.tile([128, 32 * CHUNK], _BF16, tag="set")
                for g in range(GROUPS):
                    ps = mmp.tile([128, CHUNK], _F32, tag="mm")
                    nc.tensor.matmul(
                        ps[:], AT[:, g * 128:(g + 1) * 128],
                        xT[:, g * CHUNK:(g + 1) * CHUNK],
                        start=True, stop=True)
                    if g % 2:
                        nc.scalar.copy(z1[:, g * CHUNK:(g + 1) * CHUNK], ps[:])
                    else:
                        nc.vector.tensor_copy(
                            z1[:, g * CHUNK:(g + 1) * CHUNK], ps[:])

                def zsl(t):
                    return z1[:, t * CHUNK:(t + 1) * CHUNK]

                # ---- S2: butterfly1: up t | um 8+t | rup 16+s | rum 24+s
                s2 = bigp.tile([128, 32 * CHUNK], _BF16, tag="set")

                def s2sl(i):
                    return s2[:, i * CHUNK:(i + 1) * CHUNK]

                for t in range(8):
                    nc.vector.tensor_add(s2sl(t), zsl(t), zsl(31 - t))
                    nc.vector.tensor_sub(s2sl(8 + t), zsl(t), zsl(31 - t))
                for i, sx in enumerate(range(8, 16)):
                    nc.gpsimd.tensor_add(s2sl(16 + i), zsl(sx), zsl(31 - sx))
                    nc.gpsimd.tensor_sub(s2sl(24 + i), zsl(sx), zsl(31 - sx))

                # ---- S3/S4: q0 t | q1 8+t | a 16+t | bt 24+t
                qs = bigp.tile([128, 32 * CHUNK], _BF16, tag="set")

                def qsl(i):
                    return qs[:, i * CHUNK:(i + 1) * CHUNK]

                for t in range(8):
                    nc.vector.tensor_add(qsl(t), s2sl(t), s2sl(16 + 7 - t))
                    nc.vector.tensor_sub(qsl(8 + t), s2sl(t), s2sl(16 + 7 - t))
                for t in range(8):
                    umt, rumt = s2sl(8 + t), s2sl(24 + 7 - t)
                    tn = fco[:, 0 + t:1 + t]
                    cs = fco[:, 8 + t:9 + t]
                    ns = fco[:, 16 + t:17 + t]
                    tmp = scrp.tile([128, CHUNK], _BF16, tag="scr")
                    nc.vector.scalar_tensor_tensor(
                        tmp[:], rumt, tn, umt, op0=_MULT, op1=_ADD)
                    nc.vector.tensor_scalar_mul(qsl(16 + t), tmp[:], cs)
                    tmp2 = scrp.tile([128, CHUNK], _BF16, tag="scr")
                    nc.vector.scalar_tensor_tensor(
                        tmp2[:], umt, tn, rumt, op0=_MULT, op1=_SUB)
                    nc.vector.tensor_scalar_mul(qsl(24 + t), tmp2[:], ns)

                # ---- S5: fwd leaves. comb: A'nat t | B'nat 8+t | A'sh 16+t
                # | B'dn 24+t ; z2: G0 t | G1 8+t | G2 16+t | G3 24+t
                comb = bigp.tile([128, 32 * CHUNK], _BF16, tag="set")
                z2 = bigp.tile([128, 32 * CHUNK], _BF16, tag="set")

                def csl(i):
                    return comb[:, i * CHUNK:(i + 1) * CHUNK]

                def z2sl(i):
                    return z2[:, i * CHUNK:(i + 1) * CHUNK]

                for tau in range(8):
                    w1s = wfp.tile([128, 8, 128], _BF16, tag="wf")
                    nc.sync.dma_start(w1s[:], W1d[:, tau])
                    psG0 = mmp.tile([128, CHUNK], _F32, tag="mm")
                    for t in range(8):
                        nc.tensor.matmul(psG0[:], w1s[:, t, :], qsl(t),
                                         start=(t == 0), stop=(t == 7))
                    nc.scalar.copy(z2sl(tau), psG0[:])
                    psA = mmp.tile([128, CHUNK], _F32, tag="mm")
                    for t in range(8):
                        nc.tensor.matmul(psA[:], w1s[:, t, :], qsl(16 + t),
                                         start=(t == 0), stop=(t == 7))
                    nc.vector.tensor_copy(csl(tau), psA[:])
                for tau in range(8):
                    w2s = wfp.tile([128, 8, 128], _BF16, tag="wf")
                    nc.sync.dma_start(w2s[:], W2d[:, tau])
                    psG1 = mmp.tile([128, CHUNK], _F32, tag="mm")
                    for t in range(8):
                        nc.tensor.matmul(psG1[:], w2s[:, t, :], qsl(8 + t),
                                         start=(t == 0), stop=(t == 7))
                    nc.scalar.copy(z2sl(8 + tau), psG1[:])
                for tau in range(8):
                    w1rs = wfp.tile([128, 8, 128], _BF16, tag="wf")
                    nc.sync.dma_start(w1rs[:], W1rd[:, tau])
                    psB = mmp.tile([128, CHUNK], _F32, tag="mm")
                    for t in range(8):
                        nc.tensor.matmul(psB[:], w1rs[:, t, :], qsl(24 + t),
                                         start=(t == 0), stop=(t == 7))
                    nc.vector.tensor_copy(csl(8 + tau), psB[:])

                # ---- S6: partition shifts via SBUF->SBUF DMA + combine
                for tau in range(8):
                    # A'sh[tau][0:127] = A'nat[tau][1:128]
                    nc.gpsimd.dma_start(csl(16 + tau)[0:127, :],
                                        csl(tau)[1:128, :])
                    if tau < 7:
                        nc.gpsimd.dma_start(csl(16 + tau)[127:128, :],
                                            csl(tau + 1)[0:1, :])
                    else:
                        nc.gpsimd.dma_start(csl(16 + tau)[127:128, :], zrow[:])
                    # B'dn[tau][1:128] = B'nat[tau][0:127]
                    nc.gpsimd.dma_start(csl(24 + tau)[1:128, :],
                                        csl(8 + tau)[0:127, :])
                    if tau > 0:
                        nc.gpsimd.dma_start(csl(24 + tau)[0:1, :],
                                            csl(8 + tau - 1)[127:128, :])
                    else:
                        nc.gpsimd.dma_start(csl(24 + tau)[0:1, :], zrow[:])
                for tau in range(8):
                    nc.vector.tensor_add(z2sl(16 + tau), csl(tau),
                                         csl(24 + tau))
                    nc.vector.tensor_sub(z2sl(24 + tau), csl(16 + tau),
                                         csl(8 + tau))

                # ---- S7 gconvD + S9 rotations
                z3 = bigp.tile([128, 32 * CHUNK], _BF16, tag="set")

                def z3sl(i):
                    return z3[:, i * CHUNK:(i + 1) * CHUNK]

                def dmm(r, slot, rhs_tau):
                    ps = mmp.tile([128, CHUNK], _F32, tag="mm")
                    for rp in range(4):
                        nc.tensor.matmul(
                            ps[:],
                            DT[r][rp][:, slot * 128:(slot + 1) * 128],
                            z2sl(rp * 8 + rhs_tau),
                            start=(rp == 0), stop=(rp == 3))
                    return ps

                for tau in range(8):
                    ps = dmm(0, tau, tau)
                    (nc.scalar.copy if tau % 2 else nc.vector.tensor_copy)(
                        z3sl(tau), ps[:])
                for tau in range(8):
                    ps = dmm(1, tau, tau)
                    (nc.scalar.copy if tau % 2 else nc.vector.tensor_copy)(
                        z3sl(8 + tau), ps[:])
                # odd-branch: ah e 16+tg | ah o 20+tg | bh e 24+tg | bh o 28+tg
                for tg in range(4):
                    g2n_ps = dmm(2, tg, tg)       # G2nat[tg]
                    g3r = dmm(3, 4 + tg, 7 - tg)  # G3rev[tg]
                    g2n = scrp.tile([128, CHUNK], _BF16, tag="scp")
                    nc.scalar.copy(g2n[:], g2n_ps[:])
                    te = ico[:, 0 + tg:1 + tg]
                    ce = ico[:, 4 + tg:5 + tg]
                    ne = ico[:, 8 + tg:9 + tg]
                    tmp = scrp.tile([128, CHUNK], _BF16, tag="scr")
                    nc.vector.scalar_tensor_tensor(
                        tmp[:], g3r[:], te, g2n[:], op0=_MULT, op1=_ADD)
                    nc.vector.tensor_scalar_mul(z3sl(16 + tg), tmp[:], ce)
                    tmp2 = scrp.tile([128, CHUNK], _BF16, tag="scr")
                    nc.vector.scalar_tensor_tensor(
                        tmp2[:], g2n[:], te, g3r[:], op0=_MULT, op1=_SUB)
                    nc.vector.tensor_scalar_mul(z3sl(24 + tg), tmp2[:], ne)
                    g3n_ps = dmm(3, tg, tg)       # G3nat[tg]
                    g2r = dmm(2, 4 + tg, 7 - tg)  # G2rev[tg]
                    g3n = scrp.tile([128, CHUNK], _BF16, tag="scp")
                    nc.scalar.copy(g3n[:], g3n_ps[:])
                    to = ico[:, 12 + tg:13 + tg]
                    co = ico[:, 16 + tg:17 + tg]
                    so = ico[:, 20 + tg:21 + tg]
                    tmp3 = scrp.tile([128, CHUNK], _BF16, tag="scr")
                    nc.vector.scalar_tensor_tensor(
                        tmp3[:], g2r[:], to, g3n[:], op0=_MULT, op1=_ADD)
                    nc.vector.tensor_scalar_mul(z3sl(20 + tg), tmp3[:], co)
                    tmp4 = scrp.tile([128, CHUNK], _BF16, tag="scr")
                    nc.vector.scalar_tensor_tensor(
                        tmp4[:], g3n[:], to, g2r[:], op0=_MULT, op1=_SUB)
                    nc.vector.tensor_scalar_mul(z3sl(28 + tg), tmp4[:], so)

                # ---- S8: inverse-even leaves, nt-pairs; su/df -> s11 pool
                sudf = {}
                for pair in ((0, 1), (2, 3)):
                    for s in range(2):
                        psq = {}
                        for nt in pair:
                            for br, wd, bidx in ((0, W3d, 0), (1, W2id, 1)):
                                ps = ivp.tile([128, CHUNK], _F32, tag="iv")
                                nc.tensor.matmul(
                                    ps[:], ones1[0:1, 0:128],
                                    brows[0:1, bidx * H + 512 * s:
                                          bidx * H + 512 * (s + 1)],
                                    start=True, stop=False)
                                psq[(nt, br)] = ps
                        for tau in range(8):
                            for br, wd in ((0, W3d), (1, W2id)):
                                ws = wip.tile([128, CHUNK], _BF16, tag="wi")
                                nc.sync.dma_start(
                                    ws[:], wd[:, tau, 512 * s:512 * (s + 1)])
                                for nt in pair:
                                    nc.tensor.matmul(
                                        psq[(nt, br)][:],
                                        z3sl(8 * br + tau)[
                                            :, nt * 128:(nt + 1) * 128],
                                        ws[:],
                                        start=False, stop=(tau == 7))
                        for nt in pair:
                            c1 = scrp.tile([128, CHUNK], _BF16, tag="scp")
                            nc.scalar.copy(c1[:], psq[(nt, 1)][:])
                            su = s11p.tile([128, CHUNK], _BF16,
                                           tag=f"su{nt % 2}{s}", name="su")
                            df = s11p.tile([128, CHUNK], _BF16,
                                           tag=f"df{nt % 2}{s}", name="df")
                            nc.vector.tensor_add(
                                su[:], psq[(nt, 0)][:], c1[:])
                            nc.vector.tensor_sub(
                                df[:], psq[(nt, 0)][:], c1[:])
                            sudf[(nt, s)] = (su, df)

                    # ---- S10 + S11 per nt of this pair
                    for nt in pair:
                        pAP = {}
                        for key, bidx in (("A0", 2), ("A1", 2),
                                          ("P0", 3), ("P1", 3)):
                            s = int(key[1])
                            ps = ivp.tile([128, CHUNK], _F32, tag="iv")
                            nc.tensor.matmul(
                                ps[:], ones1[0:1, 0:128],
                                brows[0:1, bidx * H + 512 * s:
                                      bidx * H + 512 * (s + 1)],
                                start=True, stop=False)
                            pAP[key] = ps
                        for tg in range(8):
                            for s in range(2):
                                wsl = W4r[:, tg, 4 * s:4 * s + 4, :]
                                nc.tensor.matmul(
                                    pAP[f"A{s}"][:],
                                    z3sl(16 + tg)[:, nt * 128:(nt + 1) * 128],
                                    wsl, start=False, stop=(tg == 7))
                                nc.tensor.matmul(
                                    pAP[f"P{s}"][:],
                                    z3sl(24 + tg)[:, nt * 128:(nt + 1) * 128],
                                    wsl, start=False, stop=(tg == 7))
                        A0, A1 = pAP["A0"], pAP["A1"]
                        P0 = s11p.tile([128, CHUNK], _BF16, tag="pp0")
                        P1 = s11p.tile([128, CHUNK], _BF16, tag="pp1")
                        nc.scalar.copy(P0[:], pAP["P0"][:])
                        nc.scalar.copy(P1[:], pAP["P1"][:])
                        es0 = s11p.tile([128, CHUNK], _BF16, tag="es0")
                        es1 = s11p.tile([128, CHUNK], _BF16, tag="es1")
                        os0 = s11p.tile([128, CHUNK], _BF16, tag="os0")
                        os1 = s11p.tile([128, CHUNK], _BF16, tag="os1")
                        nc.vector.tensor_copy(es0[:, 0:1], A0[:, 0:1])
                        nc.vector.tensor_add(
                            es0[:, 1:512], A0[:, 1:512], P1[:, 511:0:-1])
                        nc.vector.tensor_add(
                            es1[:, 0:1], A1[:, 0:1], P1[:, 0:1])
                        nc.vector.tensor_add(
                            es1[:, 1:512], A1[:, 1:512], P0[:, 511:0:-1])
                        nc.vector.tensor_sub(
                            os0[:, 0:511], A0[:, 1:512], P1[:, 511:0:-1])
                        nc.vector.tensor_sub(
                            os0[:, 511:512], A1[:, 0:1], P1[:, 0:1])
                        nc.vector.tensor_sub(
                            os1[:, 0:511], A1[:, 1:512], P0[:, 511:0:-1])
                        nc.vector.tensor_scalar_mul(
                            os1[:, 511:512], P0[:, 0:1], -1.0)
                        rows = slice(r0 + nt * 128, r0 + (nt + 1) * 128)
                        for fs in range(4):
                            if fs < 2:
                                su, _ = sudf[(nt, fs)]
                                upe = su[:, 0::2]
                                upo = su[:, 1::2]
                            else:
                                _, df = sudf[(nt, 1 if fs == 2 else 0)]
                                upe = df[:, 511:0:-2]
                                upo = df[:, 510::-2]
                            es_t = es0 if fs < 2 else es1
                            os_t = os0 if fs < 2 else os1
                            half = slice(256 * (fs % 2), 256 * (fs % 2) + 256)
                            ope = oop.tile([128, 256], _F32, tag="oo")
                            opo = oop.tile([128, 256], _F32, tag="oo")
                            ome = oop.tile([128, 256], _F32, tag="oo")
                            omo = oop.tile([128, 256], _F32, tag="oo")
                            nc.vector.tensor_add(ope[:], upe, es_t[:, half])
                            nc.vector.tensor_add(opo[:], upo, os_t[:, half])
                            nc.gpsimd.tensor_sub(ome[:], upe, es_t[:, half])
                            nc.gpsimd.tensor_sub(omo[:], upo, os_t[:, half])
                            nc.sync.dma_start(
                                out[rows, 256 * fs:256 * fs + 256], ope[:])
                            nc.sync.dma_start(
                                out[rows, 2048 + 256 * fs:2048 + 256 * fs + 256],
                                opo[:])
                            nc.sync.dma_start(
                                out[rows, 3840 - 256 * fs:4096 - 256 * fs],
                                ome[:, ::-1])
                            nc.sync.dma_start(
                                out[rows, 1792 - 256 * fs:2048 - 256 * fs],
                                omo[:, ::-1])
            if rep_ctx is not None:
                rep_ctx.__exit__(None, None, None)
    nc.finalize()
    return nc


_CACHE = {}


def _make_in_maps(x, A, D, bias, consts):
    c = consts
    x = np.ascontiguousarray(x, dtype=np.float32)
    common = {
        "Aw": np.ascontiguousarray(A, dtype=np.float32),
        "Dw": np.ascontiguousarray(D, dtype=np.float32),
        "W1d": c["W1"], "W1rd": c["W1r"], "W2d": c["W2"],
        "W3d": c["W3"], "W2id": c["W2i"], "W4d": c["W4"],
        "fcoefd": c["fcoef"], "icoefd": c["icoef"], "browsd": c["brows"],
    }
    in_maps = []
    for cc in range(N_CORES):
        m = dict(common)
        m["xs"] = x[cc * N_SHARD:(cc + 1) * N_SHARD]
        in_maps.append(m)
    return in_maps


def kernel(x, A, D, bias):
    if "nc" not in _CACHE:
        _CACHE["nc"] = _build_program()
    consts = _host_constants(bias)
    in_maps = _make_in_maps(x, A, D, bias, consts)
    res = run_bass_kernel_spmd(_CACHE["nc"], in_maps, core_ids=list(range(N_CORES)))
    return np.concatenate([res.results[cc]["out"] for cc in range(N_CORES)], axis=0)


# revision 4
# speedup vs baseline: 34.0304x; 34.0304x over previous
"""Trainium2 Bass kernel for BlockDiagonalACDC — level-2 factored DCT.

out = riffle(idct2(gconvD(dct2(gconvA(x))))) + bias, with both dense DCT
passes factored one radix-2 level deeper than the parity split:
  II_4096 -> bfly -> [II_2048 -> bfly -> II_1024 (+) IV_1024,
                      IV_2048 -> Givens rot -> II_1024 (+) II_1024-rev
                                 -> partition-shift combine]
Leaves are 1024^2 dense matmuls (4 per pass vs 2x 2048^2): dense PE work
halves per pass.  gconvD is conjugated into the level-2 permuted basis
(quadrant tiles with 32-sub-block diagonals, built on device from D).
The inverse pass uses C_IV^T = C_IV so its rotations also act on the
partition (frequency) side; shift-combines land on the free dim there.
Bias is injected via K=1 PSUM-init matmuls with host-solved preimage rows;
riffle is folded into the output DMA column maps.

Sharding: pure data parallel, 2048 rows per core on 8 cores.
"""

import numpy as np
import ml_dtypes

import concourse.bacc as bacc
import concourse.mybir as mybir
from concourse.tile import TileContext
from concourse.bass_utils import run_bass_kernel_spmd
from concourse.masks import make_identity

N_BATCH, D_FEAT, GROUPS = 16384, 4096, 32
N_CORES = 8
N_SHARD = N_BATCH // N_CORES      # 2048 rows per core
CHUNK = 512
N_CHUNKS = N_SHARD // CHUNK       # 4
H = 1024
H2 = 2048
ALPHA = [0, 2, 1, 3]              # k mod 4 for groups G0..G3

_BF16 = mybir.dt.bfloat16
_F32 = mybir.dt.float32
_MULT = mybir.AluOpType.mult
_ADD = mybir.AluOpType.add
_SUB = mybir.AluOpType.subtract


# ---------------------------------------------------------------------------
# host constants (validated in newconsts.py)
# ---------------------------------------------------------------------------

def _host_constants(bias):
    j1 = np.arange(H)
    II = 2.0 * np.cos(np.pi * np.arange(H)[:, None] * (2 * j1 + 1) / (2 * H))
    IV2 = 2.0 * np.cos(
        np.pi * (2 * np.arange(H)[:, None] + 1) * (2 * j1 + 1) / (4 * H))
    PHI = np.pi * (2 * np.arange(H2) + 1) / (4 * H2)
    s = 1.0 / (2.0 * D_FEAT)
    bf = ml_dtypes.bfloat16

    def lhsT4(M):   # [pj, tau, tj, pm] = M[128tau+pm, 128tj+pj]
        return np.ascontiguousarray(
            M.reshape(8, 128, 8, 128).transpose(3, 0, 2, 1)).astype(bf)

    c = {}
    c["W1"] = lhsT4(II)
    c["W1r"] = lhsT4(II[::-1, :])
    c["W2"] = lhsT4(IV2)
    IIw = II.copy()
    IIw[0, :] *= 0.5                               # diag(w) k=0
    c["W3"] = np.ascontiguousarray(
        (s * IIw).reshape(8, 128, H).transpose(1, 0, 2)).astype(bf)
    c["W2i"] = np.ascontiguousarray(
        (s * IV2).reshape(8, 128, H).transpose(1, 0, 2)).astype(bf)
    kap = np.empty(H, dtype=np.int64)
    kap[:512] = 2 * np.arange(512)
    kap[512:] = 2 * np.arange(512) + 1
    c["W4"] = np.ascontiguousarray(
        (s * II.T[kap, :]).reshape(8, 128, 8, 128).transpose(1, 0, 2, 3)
    ).astype(bf)

    # coeffs: fcoef [128, 24] = (tan|cos|nsc) x 8 fwd tiles;
    # icoef [128, 12]: tan_e, cos_e, nsc_e(=-cos_e) tiles 0..3? packed below
    jv = np.arange(H)
    fco = np.stack([np.tan(PHI[jv]), np.cos(PHI[jv]),
                    -((-1.0) ** jv) * np.cos(PHI[jv])])     # [3, 1024]
    c["fcoef"] = np.ascontiguousarray(
        fco.reshape(3, 8, 128).transpose(2, 0, 1).reshape(128, 24)
    ).astype(np.float32)                 # col = 8*which + t
    m5 = np.arange(512)
    ke, ko = 2 * m5, 2 * m5 + 1
    ico = np.stack([np.tan(PHI[ke]), np.cos(PHI[ke]), -np.cos(PHI[ke]),
                    np.tan(PHI[ko]), np.cos(PHI[ko]), np.cos(PHI[ko])])
    c["icoef"] = np.ascontiguousarray(
        ico.reshape(6, 4, 128).transpose(2, 0, 1).reshape(128, 24)
    ).astype(np.float32)                 # col = 4*which + tg

    # bias rows
    bias = np.asarray(bias, np.float64).reshape(-1)
    f = np.arange(H2)
    rcol = np.where(f % 2 == 0, f // 2, H2 + (f - 1) // 2)
    rcolr = np.where((D_FEAT - 1 - f) % 2 == 0, (D_FEAT - 1 - f) // 2,
                     H2 + (D_FEAT - 2 - f) // 2)
    biasf = bias[rcol]
    biasr = bias[rcolr]
    t_up = (biasf + biasr) / 2.0
    t_um = (biasf - biasr) / 2.0
    jj = np.arange(H)
    b_q0 = (t_up[jj] + t_up[H2 - 1 - jj]) / 2.0
    b_q1 = (t_up[jj] - t_up[H2 - 1 - jj]) / 2.0
    t_ev, t_od = t_um[0::2], t_um[1::2]
    alpha_v = np.empty(H)
    alpha_v[0] = t_ev[0]
    alpha_v[1:] = (t_od[:-1] + t_ev[1:]) / 2.0
    beta = np.zeros(H + 1)
    beta[1:H] = (t_ev[1:] - t_od[:-1]) / 2.0
    beta[H] = -t_od[H - 1]
    b_P = beta[H - np.arange(H)]
    c["brows"] = np.concatenate(
        [b_q0, b_q1, alpha_v, b_P])[None].astype(bf)   # [1, 4096]
    return c


def _neg(hi, step):
    """stop index for a negative-step slice ending at 0 inclusive."""
    return slice(hi, None, step)


def _build_program(reps=1):
    nc = bacc.Bacc()
    xs = nc.dram_tensor("xs", (N_SHARD, D_FEAT), _F32, kind="ExternalInput")
    Aw = nc.dram_tensor("Aw", (GROUPS, 128, 128), _F32, kind="ExternalInput")
    Dw = nc.dram_tensor("Dw", (GROUPS, 128, 128), _F32, kind="ExternalInput")
    W1d = nc.dram_tensor("W1d", (128, 8, 8, 128), _BF16, kind="ExternalInput")
    W1rd = nc.dram_tensor("W1rd", (128, 8, 8, 128), _BF16, kind="ExternalInput")
    W2d = nc.dram_tensor("W2d", (128, 8, 8, 128), _BF16, kind="ExternalInput")
    W3d = nc.dram_tensor("W3d", (128, 8, H), _BF16, kind="ExternalInput")
    W2id = nc.dram_tensor("W2id", (128, 8, H), _BF16, kind="ExternalInput")
    W4d = nc.dram_tensor("W4d", (128, 8, 8, 128), _BF16, kind="ExternalInput")
    fcoefd = nc.dram_tensor("fcoefd", (128, 24), _F32, kind="ExternalInput")
    icoefd = nc.dram_tensor("icoefd", (128, 24), _F32, kind="ExternalInput")
    browsd = nc.dram_tensor("browsd", (1, 4 * H), _BF16, kind="ExternalInput")
    out = nc.dram_tensor("out", (N_SHARD, D_FEAT), _F32, kind="ExternalOutput")

    with TileContext(nc) as tc:
        with (
            tc.tile_pool(name="const", bufs=1) as constp,
            tc.tile_pool(name="big", bufs=3) as bigp,
            tc.tile_pool(name="xbf", bufs=2) as xbfp,
            tc.tile_pool(name="wf", bufs=3) as wfp,
            tc.tile_pool(name="wi", bufs=3) as wip,
            tc.tile_pool(name="scr", bufs=2) as scrp,
            tc.tile_pool(name="s11", bufs=1) as s11p,
            tc.tile_pool(name="oo", bufs=4) as oop,
            tc.tile_pool(name="tp_ps", bufs=1, space="PSUM") as tpp,
            tc.tile_pool(name="mm_ps", bufs=3, space="PSUM") as mmp,
            tc.tile_pool(name="iv_ps", bufs=4, space="PSUM") as ivp,
        ):
            ident = constp.tile([128, 128], _BF16, tag="ident")
            make_identity(nc, ident[:])
            ones1 = constp.tile([1, 128], _BF16, tag="ones1")
            nc.gpsimd.memset(ones1[:], 1.0)
            zrow = constp.tile([1, 512], _BF16, tag="zrow")
            nc.gpsimd.memset(zrow[:], 0.0)
            fco = constp.tile([128, 24], _F32, tag="fco")
            ico = constp.tile([128, 24], _F32, tag="ico")
            brows = constp.tile([1, 4 * H], _BF16, tag="brows")
            nc.gpsimd.dma_start(fco[:], fcoefd[:])
            nc.gpsimd.dma_start(ico[:], icoefd[:])
            nc.gpsimd.dma_start(brows[:], browsd[:])
            W4r = constp.tile([128, 8, 8, 128], _BF16, tag="W4r")
            nc.sync.dma_start(W4r[:], W4d[:])

            # ---- A weights: AT[g] = A[g].T, partition-reversed output for
            # groups with (g//8) odd (free-reversed copy)
            AT = constp.tile([128, D_FEAT], _BF16, tag="AT")
            for hf in range(2):
                awbf = xbfp.tile([128, D_FEAT // 2], _BF16, tag="xbf")
                for gl in range(16):
                    g = hf * 16 + gl
                    nc.gpsimd.dma_start(awbf[:, gl * 128:(gl + 1) * 128], Aw[g])
                for g4 in range(4):
                    ps = tpp.tile([128, 512], _BF16, tag="tp")
                    for gg in range(4):
                        gl = g4 * 4 + gg
                        nc.tensor.transpose(
                            ps[:, gg * 128:(gg + 1) * 128],
                            awbf[:, gl * 128:(gl + 1) * 128], ident[:])
                    for gg in range(4):
                        g = hf * 16 + g4 * 4 + gg
                        sl = slice(g * 128, (g + 1) * 128)
                        if (g // 8) % 2 == 0:
                            nc.vector.tensor_copy(
                                AT[:, sl], ps[:, gg * 128:(gg + 1) * 128])
                        else:
                            nc.vector.tensor_copy(
                                AT[:, sl],
                                ps[:, (gg + 1) * 128 - 1:
                                   gg * 128 - 1 if gg else None:-1])

            # ---- conjugated D tiles: DTall[r][rp] = [128, 8 slots x 128]
            # r<2: slots 0..7 nat tau; r>=2: slots 0..3 nat tau 0..3,
            # slots 4..7 = reversed emissions (out tile tau'=slot-4, content
            # from original tau = 11-slot, consumes rhs tile 11-slot)
            DT = [[constp.tile([128, 8 * 128], _BF16, tag=f"DT{r}{rp}",
                               name=f"DT{r}{rp}")
                   for rp in range(4)] for r in range(4)]
            for r in range(4):
                for rp in range(4):
                    nc.gpsimd.memset(DT[r][rp][:], 0.0)
            dwh = [None, None]
            for hf in range(2):
                dwh[hf] = xbfp.tile([128, D_FEAT // 2], _BF16, tag="xbf",
                                    name="dwbf")
                for gl in range(16):
                    nc.gpsimd.dma_start(
                        dwh[hf][:, gl * 128:(gl + 1) * 128], Dw[hf * 16 + gl])
            for tau in range(8):
                dwbf0 = dwh[tau // 4]
                off = 512 * (tau % 4)
                for rp in range(4):
                    ps = tpp.tile([128, 512], _BF16, tag="tp")
                    nc.tensor.transpose(
                        ps[:, 0:128],
                        dwbf0[:, off + ALPHA[rp]:off + 512:4],
                        ident[:])
                    for r in range(4):
                        a0 = ALPHA[r]
                        if r < 2 or tau < 4:
                            base = tau * 128
                            for u in range(4):
                                nc.vector.tensor_copy(
                                    DT[r][rp][32 * u:32 * u + 32,
                                              base + 32 * u:base + 32 * u + 32],
                                    ps[32 * u:32 * u + 32, a0:a0 + 128:4])
                        if r >= 2 and tau >= 4:
                            base = (4 + (7 - tau)) * 128
                            for u in range(4):
                                nc.vector.tensor_copy(
                                    DT[r][rp][32 * u:32 * u + 32,
                                              base + 32 * (3 - u):
                                              base + 32 * (3 - u) + 32],
                                    ps[32 * u:32 * u + 32, a0 + 124::-4])

            rep_ctx = tc.For_i(0, reps, 1) if reps > 1 else None
            if rep_ctx is not None:
                rep_ctx.__enter__()
            for ci in range(N_CHUNKS):
                r0 = ci * CHUNK
                # ---- S0: transpose-in -> xT (big pool)
                xT = bigp.tile([128, 32 * CHUNK], _BF16, tag="set")
                for ntp in range(2):
                    for hf in range(2):
                        xbfs = []
                        for nn in range(2):
                            nt = ntp * 2 + nn
                            xbf = xbfp.tile([128, D_FEAT // 2], _BF16,
                                            tag="xbf")
                            nc.gpsimd.dma_start(
                                xbf[:], xs[r0 + nt * 128:r0 + (nt + 1) * 128,
                                           hf * 2048:(hf + 1) * 2048])
                            xbfs.append(xbf)
                        for fl in range(16):
                            fc = hf * 16 + fl
                            ps = tpp.tile([128, 512], _BF16, tag="tp")
                            for nn in range(2):
                                nc.tensor.transpose(
                                    ps[:, nn * 128:(nn + 1) * 128],
                                    xbfs[nn][:, fl * 128:(fl + 1) * 128],
                                    ident[:])
                            eng = nc.vector if fc % 2 else nc.scalar
                            (eng.tensor_copy if eng is nc.vector else eng.copy)(
                                xT[:, fc * CHUNK + ntp * 256:
                                   fc * CHUNK + ntp * 256 + 256],
                                ps[:, 0:256])
                # ---- S1: gconvA (32 MMs), identity slots
                z1 = bigp.tile([128, 32 * CHUNK], _BF16, tag="set")
                for g in range(GROUPS):
                    ps = mmp.tile([128, CHUNK], _F32, tag="mm")
                    nc.tensor.matmul(
                        ps[:], AT[:, g * 128:(g + 1) * 128],
                        xT[:, g * CHUNK:(g + 1) * CHUNK],
                        start=True, stop=True)
                    if g % 2:
                        nc.scalar.copy(z1[:, g * CHUNK:(g + 1) * CHUNK], ps[:])
                    else:
                        nc.vector.tensor_copy(
                            z1[:, g * CHUNK:(g + 1) * CHUNK], ps[:])

                def zsl(t):
                    return z1[:, t * CHUNK:(t + 1) * CHUNK]

                # ---- S2: butterfly1: up t | um 8+t | rup 16+s | rum 24+s
                s2 = bigp.tile([128, 32 * CHUNK], _BF16, tag="set")

                def s2sl(i):
                    return s2[:, i * CHUNK:(i + 1) * CHUNK]

                for t in range(8):
                    nc.vector.tensor_add(s2sl(t), zsl(t), zsl(31 - t))
                    nc.vector.tensor_sub(s2sl(8 + t), zsl(t), zsl(31 - t))
                for i, sx in enumerate(range(8, 16)):
                    nc.gpsimd.tensor_add(s2sl(16 + i), zsl(sx), zsl(31 - sx))
                    nc.gpsimd.tensor_sub(s2sl(24 + i), zsl(sx), zsl(31 - sx))

                # ---- S3/S4: q0 t | q1 8+t | a 16+t | bt 24+t
                qs = bigp.tile([128, 32 * CHUNK], _BF16, tag="set")

                def qsl(i):
                    return qs[:, i * CHUNK:(i + 1) * CHUNK]

                for t in range(8):
                    nc.vector.tensor_add(qsl(t), s2sl(t), s2sl(16 + 7 - t))
                    nc.vector.tensor_sub(qsl(8 + t), s2sl(t), s2sl(16 + 7 - t))
                for t in range(8):
                    umt, rumt = s2sl(8 + t), s2sl(24 + 7 - t)
                    tn = fco[:, 0 + t:1 + t]
                    cs = fco[:, 8 + t:9 + t]
                    ns = fco[:, 16 + t:17 + t]
                    tmp = scrp.tile([128, CHUNK], _BF16, tag="scr")
                    nc.vector.scalar_tensor_tensor(
                        tmp[:], rumt, tn, umt, op0=_MULT, op1=_ADD)
                    nc.vector.tensor_scalar_mul(qsl(16 + t), tmp[:], cs)
                    tmp2 = scrp.tile([128, CHUNK], _BF16, tag="scr")
                    nc.vector.scalar_tensor_tensor(
                        tmp2[:], umt, tn, rumt, op0=_MULT, op1=_SUB)
                    nc.vector.tensor_scalar_mul(qsl(24 + t), tmp2[:], ns)

                # ---- S5: fwd leaves. comb: A'nat t | B'nat 8+t | A'sh 16+t
                # | B'dn 24+t ; z2: G0 t | G1 8+t | G2 16+t | G3 24+t
                comb = bigp.tile([128, 32 * CHUNK], _BF16, tag="set")
                z2 = bigp.tile([128, 32 * CHUNK], _BF16, tag="set")

                def csl(i):
                    return comb[:, i * CHUNK:(i + 1) * CHUNK]

                def z2sl(i):
                    return z2[:, i * CHUNK:(i + 1) * CHUNK]

                for tau in range(8):
                    w1s = wfp.tile([128, 8, 128], _BF16, tag="wf")
                    nc.sync.dma_start(w1s[:], W1d[:, tau])
                    psG0 = mmp.tile([128, CHUNK], _F32, tag="mm")
                    for t in range(8):
                        nc.tensor.matmul(psG0[:], w1s[:, t, :], qsl(t),
                                         start=(t == 0), stop=(t == 7))
                    nc.scalar.copy(z2sl(tau), psG0[:])
                    psA = mmp.tile([128, CHUNK], _F32, tag="mm")
                    for t in range(8):
                        nc.tensor.matmul(psA[:], w1s[:, t, :], qsl(16 + t),
                                         start=(t == 0), stop=(t == 7))
                    nc.vector.tensor_copy(csl(tau), psA[:])
                for tau in range(8):
                    w2s = wfp.tile([128, 8, 128], _BF16, tag="wf")
                    nc.sync.dma_start(w2s[:], W2d[:, tau])
                    psG1 = mmp.tile([128, CHUNK], _F32, tag="mm")
                    for t in range(8):
                        nc.tensor.matmul(psG1[:], w2s[:, t, :], qsl(8 + t),
                                         start=(t == 0), stop=(t == 7))
                    nc.scalar.copy(z2sl(8 + tau), psG1[:])
                for tau in range(8):
                    w1rs = wfp.tile([128, 8, 128], _BF16, tag="wf")
                    nc.sync.dma_start(w1rs[:], W1rd[:, tau])
                    psB = mmp.tile([128, CHUNK], _F32, tag="mm")
                    for t in range(8):
                        nc.tensor.matmul(psB[:], w1rs[:, t, :], qsl(24 + t),
                                         start=(t == 0), stop=(t == 7))
                    nc.vector.tensor_copy(csl(8 + tau), psB[:])

                # ---- S6: partition shifts via SBUF->SBUF DMA + combine
                for tau in range(8):
                    # A'sh[tau][0:127] = A'nat[tau][1:128]
                    nc.gpsimd.dma_start(csl(16 + tau)[0:127, :],
                                        csl(tau)[1:128, :])
                    if tau < 7:
                        nc.gpsimd.dma_start(csl(16 + tau)[127:128, :],
                                            csl(tau + 1)[0:1, :])
                    else:
                        nc.gpsimd.dma_start(csl(16 + tau)[127:128, :], zrow[:])
                    # B'dn[tau][1:128] = B'nat[tau][0:127]
                    nc.gpsimd.dma_start(csl(24 + tau)[1:128, :],
                                        csl(8 + tau)[0:127, :])
                    if tau > 0:
                        nc.gpsimd.dma_start(csl(24 + tau)[0:1, :],
                                            csl(8 + tau - 1)[127:128, :])
                    else:
                        nc.gpsimd.dma_start(csl(24 + tau)[0:1, :], zrow[:])
                for tau in range(8):
                    nc.vector.tensor_add(z2sl(16 + tau), csl(tau),
                                         csl(24 + tau))
                    nc.vector.tensor_sub(z2sl(24 + tau), csl(16 + tau),
                                         csl(8 + tau))

                # ---- S7 gconvD + S9 rotations
                z3 = bigp.tile([128, 32 * CHUNK], _BF16, tag="set")

                def z3sl(i):
                    return z3[:, i * CHUNK:(i + 1) * CHUNK]

                def dmm(r, slot, rhs_tau):
                    ps = mmp.tile([128, CHUNK], _F32, tag="mm")
                    for rp in range(4):
                        nc.tensor.matmul(
                            ps[:],
                            DT[r][rp][:, slot * 128:(slot + 1) * 128],
                            z2sl(rp * 8 + rhs_tau),
                            start=(rp == 0), stop=(rp == 3))
                    return ps

                for tau in range(8):
                    ps = dmm(0, tau, tau)
                    (nc.scalar.copy if tau % 2 else nc.vector.tensor_copy)(
                        z3sl(tau), ps[:])
                for tau in range(8):
                    ps = dmm(1, tau, tau)
                    (nc.scalar.copy if tau % 2 else nc.vector.tensor_copy)(
                        z3sl(8 + tau), ps[:])
                # odd-branch: ah e 16+tg | ah o 20+tg | bh e 24+tg | bh o 28+tg
                for tg in range(4):
                    g2n_ps = dmm(2, tg, tg)       # G2nat[tg]
                    g3r = dmm(3, 4 + tg, 7 - tg)  # G3rev[tg]
                    g2n = scrp.tile([128, CHUNK], _BF16, tag="scp")
                    nc.scalar.copy(g2n[:], g2n_ps[:])
                    te = ico[:, 0 + tg:1 + tg]
                    ce = ico[:, 4 + tg:5 + tg]
                    ne = ico[:, 8 + tg:9 + tg]
                    tmp = scrp.tile([128, CHUNK], _BF16, tag="scr")
                    nc.vector.scalar_tensor_tensor(
                        tmp[:], g3r[:], te, g2n[:], op0=_MULT, op1=_ADD)
                    nc.vector.tensor_scalar_mul(z3sl(16 + tg), tmp[:], ce)
                    tmp2 = scrp.tile([128, CHUNK], _BF16, tag="scr")
                    nc.vector.scalar_tensor_tensor(
                        tmp2[:], g2n[:], te, g3r[:], op0=_MULT, op1=_SUB)
                    nc.vector.tensor_scalar_mul(z3sl(24 + tg), tmp2[:], ne)
                    g3n_ps = dmm(3, tg, tg)       # G3nat[tg]
                    g2r = dmm(2, 4 + tg, 7 - tg)  # G2rev[tg]
                    g3n = scrp.tile([128, CHUNK], _BF16, tag="scp")
                    nc.scalar.copy(g3n[:], g3n_ps[:])
                    to = ico[:, 12 + tg:13 + tg]
                    co = ico[:, 16 + tg:17 + tg]
                    so = ico[:, 20 + tg:21 + tg]
                    tmp3 = scrp.tile([128, CHUNK], _BF16, tag="scr")
                    nc.vector.scalar_tensor_tensor(
                        tmp3[:], g2r[:], to, g3n[:], op0=_MULT, op1=_ADD)
                    nc.vector.tensor_scalar_mul(z3sl(20 + tg), tmp3[:], co)
                    tmp4 = scrp.tile([128, CHUNK], _BF16, tag="scr")
                    nc.vector.scalar_tensor_tensor(
                        tmp4[:], g3n[:], to, g2r[:], op0=_MULT, op1=_SUB)
                    nc.vector.tensor_scalar_mul(z3sl(28 + tg), tmp4[:], so)

                # ---- S8: inverse-even leaves, nt-pairs; su/df -> s11 pool
                sudf = {}
                for pair in ((0, 1), (2, 3)):
                    for s in range(2):
                        psq = {}
                        for nt in pair:
                            for br, wd, bidx in ((0, W3d, 0), (1, W2id, 1)):
                                ps = ivp.tile([128, CHUNK], _F32, tag="iv")
                                nc.tensor.matmul(
                                    ps[:], ones1[0:1, 0:128],
                                    brows[0:1, bidx * H + 512 * s:
                                          bidx * H + 512 * (s + 1)],
                                    start=True, stop=False)
                                psq[(nt, br)] = ps
                        for tau in range(8):
                            for br, wd in ((0, W3d), (1, W2id)):
                                ws = wip.tile([128, CHUNK], _BF16, tag="wi")
                                nc.sync.dma_start(
                                    ws[:], wd[:, tau, 512 * s:512 * (s + 1)])
                                for nt in pair:
                                    nc.tensor.matmul(
                                        psq[(nt, br)][:],
                                        z3sl(8 * br + tau)[
                                            :, nt * 128:(nt + 1) * 128],
                                        ws[:],
                                        start=False, stop=(tau == 7))
                        for nt in pair:
                            c1 = scrp.tile([128, CHUNK], _BF16, tag="scp")
                            nc.scalar.copy(c1[:], psq[(nt, 1)][:])
                            su = s11p.tile([128, CHUNK], _BF16,
                                           tag=f"su{nt % 2}{s}", name="su")
                            df = s11p.tile([128, CHUNK], _BF16,
                                           tag=f"df{nt % 2}{s}", name="df")
                            nc.vector.tensor_add(
                                su[:], psq[(nt, 0)][:], c1[:])
                            nc.vector.tensor_sub(
                                df[:], psq[(nt, 0)][:], c1[:])
                            sudf[(nt, s)] = (su, df)

                    # ---- S10 + S11 per nt of this pair
                    for nt in pair:
                        pAP = {}
                        for key, bidx in (("A0", 2), ("A1", 2),
                                          ("P0", 3), ("P1", 3)):
                            s = int(key[1])
                            ps = ivp.tile([128, CHUNK], _F32, tag="iv")
                            nc.tensor.matmul(
                                ps[:], ones1[0:1, 0:128],
                                brows[0:1, bidx * H + 512 * s:
                                      bidx * H + 512 * (s + 1)],
                                start=True, stop=False)
                            pAP[key] = ps
                        for tg in range(8):
                            for s in range(2):
                                wsl = W4r[:, tg, 4 * s:4 * s + 4, :]
                                nc.tensor.matmul(
                                    pAP[f"A{s}"][:],
                                    z3sl(16 + tg)[:, nt * 128:(nt + 1) * 128],
                                    wsl, start=False, stop=(tg == 7))
                                nc.tensor.matmul(
                                    pAP[f"P{s}"][:],
                                    z3sl(24 + tg)[:, nt * 128:(nt + 1) * 128],
                                    wsl, start=False, stop=(tg == 7))
                        A0, A1 = pAP["A0"], pAP["A1"]
                        P0 = s11p.tile([128, CHUNK], _BF16, tag="pp0")
                        P1 = s11p.tile([128, CHUNK], _BF16, tag="pp1")
                        nc.scalar.copy(P0[:], pAP["P0"][:])
                        nc.scalar.copy(P1[:], pAP["P1"][:])
                        es0 = s11p.tile([128, CHUNK], _BF16, tag="es0")
                        es1 = s11p.tile([128, CHUNK], _BF16, tag="es1")
                        os0 = s11p.tile([128, CHUNK], _BF16, tag="os0")
                        os1 = s11p.tile([128, CHUNK], _BF16, tag="os1")
                        nc.vector.tensor_copy(es0[:, 0:1], A0[:, 0:1])
                        nc.vector.tensor_add(
                            es0[:, 1:512], A0[:, 1:512], P1[:, 511:0:-1])
                        nc.vector.tensor_add(
                            es1[:, 0:1], A1[:, 0:1], P1[:, 0:1])
                        nc.vector.tensor_add(
                            es1[:, 1:512], A1[:, 1:512], P0[:, 511:0:-1])
                        nc.vector.tensor_sub(
                            os0[:, 0:511], A0[:, 1:512], P1[:, 511:0:-1])
                        nc.vector.tensor_sub(
                            os0[:, 511:512], A1[:, 0:1], P1[:, 0:1])
                        nc.vector.tensor_sub(
                            os1[:, 0:511], A1[:, 1:512], P0[:, 511:0:-1])
                        nc.vector.tensor_scalar_mul(
                            os1[:, 511:512], P0[:, 0:1], -1.0)
                        rows = slice(r0 + nt * 128, r0 + (nt + 1) * 128)
                        for fs in range(4):
                            if fs < 2:
                                su, _ = sudf[(nt, fs)]
                                upe = su[:, 0::2]
                                upo = su[:, 1::2]
                            else:
                                _, df = sudf[(nt, 1 if fs == 2 else 0)]
                                upe = df[:, 511:0:-2]
                                upo = df[:, 510::-2]
                            es_t = es0 if fs < 2 else es1
                            os_t = os0 if fs < 2 else os1
                            half = slice(256 * (fs % 2), 256 * (fs % 2) + 256)
                            ope = oop.tile([128, 256], _F32, tag="oo")
                            opo = oop.tile([128, 256], _F32, tag="oo")
                            ome = oop.tile([128, 256], _F32, tag="oo")
                            omo = oop.tile([128, 256], _F32, tag="oo")
                            nc.vector.tensor_add(ope[:], upe, es_t[:, half])
                            nc.vector.tensor_add(opo[:], upo, os_t[:, half])
                            nc.gpsimd.tensor_sub(ome[:, ::-1], upe,
                                                 es_t[:, half])
                            nc.gpsimd.tensor_sub(omo[:, ::-1], upo,
                                                 os_t[:, half])
                            nc.sync.dma_start(
                                out[rows, 256 * fs:256 * fs + 256], ope[:])
                            nc.sync.dma_start(
                                out[rows, 2048 + 256 * fs:2048 + 256 * fs + 256],
                                opo[:])
                            nc.sync.dma_start(
                                out[rows, 3840 - 256 * fs:4096 - 256 * fs],
                                ome[:])
                            nc.sync.dma_start(
                                out[rows, 1792 - 256 * fs:2048 - 256 * fs],
                                omo[:])
            if rep_ctx is not None:
                rep_ctx.__exit__(None, None, None)
    nc.finalize()
    return nc


_CACHE = {}


def _make_in_maps(x, A, D, bias, consts):
    c = consts
    x = np.ascontiguousarray(x, dtype=np.float32)
    common = {
        "Aw": np.ascontiguousarray(A, dtype=np.float32),
        "Dw": np.ascontiguousarray(D, dtype=np.float32),
        "W1d": c["W1"], "W1rd": c["W1r"], "W2d": c["W2"],
        "W3d": c["W3"], "W2id": c["W2i"], "W4d": c["W4"],
        "fcoefd": c["fcoef"], "icoefd": c["icoef"], "browsd": c["brows"],
    }
    in_maps = []
    for cc in range(N_CORES):
        m = dict(common)
        m["xs"] = x[cc * N_SHARD:(cc + 1) * N_SHARD]
        in_maps.append(m)
    return in_maps


def kernel(x, A, D, bias):
    if "nc" not in _CACHE:
        _CACHE["nc"] = _build_program()
    consts = _host_constants(bias)
    in_maps = _make_in_maps(x, A, D, bias, consts)
    res = run_bass_kernel_spmd(_CACHE["nc"], in_maps, core_ids=list(range(N_CORES)))
    return np.concatenate([res.results[cc]["out"] for cc in range(N_CORES)], axis=0)
